# revision 1
# baseline (speedup 1.0000x reference)
"""Trainium2 Bass kernel for BipartiteGCN (8 NeuronCores, SPMD).

Strategy:
 - Node rows sharded 8 ways (cons: NC/8 rows per core, var: NV/8).
 - Edges sharded by DESTINATION range; per-core edges sorted by dst block
   (128 dst rows per block), bucketed by src>=32768 where needed (int16
   gather indices).
 - Per-edge pipeline: dma_gather of lp[src] and rp[dst] rows (bf16),
   add -> LayerNorm (bn_stats) -> LeakyReLU fused on ScalarE -> one-hot
   (iota is_equal) -> matmul-accumulate into PSUM per dst block: computes
   segment-sum AND counts (ones column) with no scatter DMA.
 - Linearity: wf/bias applied after the segment-mean (per node, not per
   edge).
 - Only lp tables are all-gathered; rp/skip/post-MLP/head stages stay
   local to each core's dst shard. Output is the var shard -> host concat.
"""

import os
import sys

for _p in ("/opt/trn_rl_repo",):
    if _p not in sys.path:
        sys.path.insert(0, _p)

import numpy as np
import ml_dtypes

import concourse.bass as bass
import concourse.bacc as bacc
import concourse.mybir as mybir
from concourse import tile, library_config
from concourse.bass_utils import run_bass_kernel_spmd

BF16 = ml_dtypes.bfloat16
F32 = np.float32
NCORES = 8
EMB = 128
CHUNK_TILES = 32     # tiles (128 edges) per dma_gather call (4096 edges)
HI_BASE = 32768
EPS = 1e-5
SLOPE = 0.01

dt = mybir.dt


def _wrap_idx(idx_i16):
    """[N] int16 -> [128, N//16] wrapped (i at [i%16, i//16]) + replicated 8x."""
    n = idx_i16.shape[0]
    assert n % 16 == 0
    w = idx_i16.reshape(n // 16, 16).T
    return np.tile(w, (8, 1)).copy()


def _lane_major(arr, lanes=128):
    """[N] -> [lanes, N//lanes] with element i at [i%lanes, i//lanes]."""
    n = arr.shape[0]
    assert n % lanes == 0
    return arr.reshape(n // lanes, lanes).T.copy()


def _bcast_row(v, rows=128):
    """[F] -> [rows, F] replicated, f32."""
    return np.broadcast_to(np.asarray(v, F32)[None, :], (rows, v.shape[0])).copy()


class ConvPrep:
    """Per-conv edge-sharding data. Same segment layout for all cores."""

    def __init__(self, dst, src, n_dst, n_src, dst_per_core):
        self.n_dst_local = dst_per_core
        self.nblocks = -(-dst_per_core // 128)
        self.two_buckets = n_src > HI_BASE
        nb = self.nblocks
        nu = 2 if self.two_buckets else 1

        core = dst // dst_per_core
        dloc_all = dst - core * dst_per_core
        block_all = dloc_all // 128

        # per (core, bucket, block) edge lists
        per = [[[None] * nb for _ in range(nu)] for _ in range(NCORES)]
        for c in range(NCORES):
            m = core == c
            d_c = dloc_all[m]
            s_c = src[m]
            b_c = block_all[m]
            u_c = (s_c >= HI_BASE).astype(np.int8) if self.two_buckets else np.zeros(
                len(s_c), np.int8
            )
            for u in range(nu):
                mu = u_c == u
                db, sb, bb = d_c[mu], s_c[mu], b_c[mu]
                order = np.argsort(bb, kind="stable")
                db, sb, bb = db[order], sb[order], bb[order]
                bounds = np.searchsorted(bb, np.arange(nb + 1))
                for b in range(nb):
                    lo, hi = bounds[b], bounds[b + 1]
                    per[c][u][b] = (sb[lo:hi], db[lo:hi])

        # uniform tile counts (128 edges): max over cores
        self.ntiles = np.zeros((nu, nb), np.int64)
        for u in range(nu):
            for b in range(nb):
                mx = max(len(per[c][u][b][0]) for c in range(NCORES))
                self.ntiles[u, b] = -(-mx // 128) if mx > 0 else 0

        etot = int(self.ntiles.sum()) * 128
        self.etot = etot

        # build padded per-core arrays in segment order (u-major, b-minor)
        self.src_idx = np.zeros((NCORES, etot), np.int16)
        self.dstrel = np.full((NCORES, etot), -1.0, F32)
        off = 0
        self.seg_offsets = {}
        for u in range(nu):
            for b in range(nb):
                g = int(self.ntiles[u, b])
                if g == 0:
                    continue
                self.seg_offsets[(u, b)] = off
                for c in range(NCORES):
                    sb, db = per[c][u][b]
                    n = len(sb)
                    s_adj = sb - (HI_BASE if u == 1 else 0)
                    self.src_idx[c, off : off + n] = s_adj.astype(np.int16)
                    self.dstrel[c, off : off + n] = (db - 128 * b).astype(F32)
                off += g * 128
        assert off == etot

        # stream layout: per bucket, list of (block, ntiles)
        self.streams = []
        for u in range(nu):
            blocks = [(b, int(self.ntiles[u, b])) for b in range(nb) if self.ntiles[u, b] > 0]
            start = self.seg_offsets[(u, blocks[0][0])] if blocks else 0
            nt = sum(g for _, g in blocks)
            self.streams.append({"u": u, "blocks": blocks, "start_edge": start, "ntiles": nt})

    def core_arrays(self, c):
        dr = self.dstrel[c]
        i = np.nonzero(dr >= 0)[0]
        lane = i % 128
        tb = (i // 128) * 128
        d = dr[i].astype(np.int64)
        oh = np.zeros((128, self.etot), ml_dtypes.float8_e4m3)
        oh[lane, tb + d] = 1.0
        ohT = np.zeros((128, self.etot), ml_dtypes.float8_e4m3)
        ohT[d, tb + lane] = 1.0
        return _wrap_idx(self.src_idx[c]), oh, ohT


def host_prep(inputs):
    p = {}
    cons_x = np.asarray(inputs["cons_x"], F32)
    var_x = np.asarray(inputs["var_x"], F32)
    edge_cons = np.asarray(inputs["edge_cons"]).astype(np.int64)
    edge_var = np.asarray(inputs["edge_var"]).astype(np.int64)
    head_mask = np.asarray(inputs["head_mask"]).astype(bool)

    NC, CF = cons_x.shape
    NV, VF = var_x.shape
    assert NC % NCORES == 0 and NV % NCORES == 0
    NCL, NVL = NC // NCORES, NV // NCORES
    p.update(NC=NC, NV=NV, CF=CF, VF=VF, NCL=NCL, NVL=NVL)

    # conv1: v->c (src=edge_var over NV, dst=edge_cons over NC)
    p["conv1"] = ConvPrep(edge_cons, edge_var, NC, NV, NCL)
    # conv2: c->v
    p["conv2"] = ConvPrep(edge_var, edge_cons, NV, NC, NVL)

    # ---- weights ----
    w = {}

    def embed_w(prefix, g, b, w1, b1, w2, b2, feat):
        w1 = np.asarray(w1, F32)
        w1g = np.asarray(g, F32)[:, None] * w1
        aug = np.concatenate([w1g, np.zeros((1, w1.shape[1]), F32)], 0)
        w[prefix + "w1aug"] = aug.astype(BF16)
        w[prefix + "s1"] = _bcast_row(np.asarray(b, F32) @ w1 + np.asarray(b1, F32))
        w[prefix + "r1"] = _bcast_row(w1g.sum(0))
        w[prefix + "w2"] = np.asarray(w2, F32).astype(BF16)
        w[prefix + "b2"] = _bcast_row(np.asarray(b2, F32))

    embed_w("ce_", inputs["ce_ln_g"], inputs["ce_ln_b"], inputs["ce_w1"],
            inputs["ce_b1"], inputs["ce_w2"], inputs["ce_b2"], CF)
    embed_w("ve_", inputs["ve_ln_g"], inputs["ve_ln_b"], inputs["ve_w1"],
            inputs["ve_b1"], inputs["ve_w2"], inputs["ve_b2"], VF)

    for pre in ("vc_", "cv_"):
        wl = np.asarray(inputs[pre + "wl"], F32)
        w[pre + "wl"] = wl.astype(BF16)
        w[pre + "bl"] = _bcast_row(np.asarray(inputs[pre + "bl"], F32))
        w[pre + "wr"] = np.asarray(inputs[pre + "wr"], F32).astype(BF16)
        flg = np.asarray(inputs[pre + "flg"], F32)
        flb = np.asarray(inputs[pre + "flb"], F32)
        p[pre + "fl_trivial"] = bool(np.all(flg == 1.0) and np.all(flb == 0.0))
        w[pre + "flg"] = _bcast_row(flg)
        w[pre + "flb"] = _bcast_row(flb)
        w[pre + "wf"] = np.asarray(inputs[pre + "wf"], F32).astype(BF16)
        w[pre + "bf"] = _bcast_row(np.asarray(inputs[pre + "bf"], F32))
        wo1 = np.asarray(inputs[pre + "wo1"], F32)
        plg = np.asarray(inputs[pre + "plg"], F32)
        plb = np.asarray(inputs[pre + "plb"], F32)
        w[pre + "wo1a"] = (plg[:, None] * wo1[:EMB]).astype(BF16)
        w[pre + "wo1b"] = wo1[EMB:].astype(BF16)
        w[pre + "bo1"] = _bcast_row(np.asarray(inputs[pre + "bo1"], F32) + plb @ wo1[:EMB])
        w[pre + "wo2"] = np.asarray(inputs[pre + "wo2"], F32).astype(BF16)
        w[pre + "bo2"] = _bcast_row(np.asarray(inputs[pre + "bo2"], F32))

    # heads
    active = np.nonzero(head_mask)[0]
    nact = int(len(active))
    p["nact"] = nact
    denom = max(float(head_mask.sum()), 1.0)
    hb2 = np.asarray(inputs["hb2"], F32)
    p["out_scale"] = 1.0 / denom
    p["out_add"] = float(hb2[active].sum() / denom)
    if nact > 0:
        hw1 = np.asarray(inputs["hw1"], F32)[active]          # [nact,128,128]
        w["hw1"] = hw1.transpose(1, 0, 2).astype(BF16).copy()  # [128,nact,128]
        w["hb1"] = np.asarray(inputs["hb1"], F32)[active].T.copy()   # [128,nact]
        w["hw2"] = np.asarray(inputs["hw2"], F32)[active].T.astype(BF16).copy()  # [128,nact]

    w["identity"] = np.eye(128, dtype=BF16)
    p["weights"] = w

    # ---- per-core inputs ----
    NCLp = -(-NCL // 128) * 128
    NVLp = -(-NVL // 128) * 128
    p.update(NCLp=NCLp, NVLp=NVLp)
    NVLh = -(-NVL // 512) * 512  # head stage col padding
    p["NVLh"] = NVLh

    core_inputs = []
    for c in range(NCORES):
        m = {}
        cx = cons_x[c * NCL : (c + 1) * NCL]
        vx = var_x[c * NVL : (c + 1) * NVL]
        cxp = np.zeros((NCLp, CF), F32)
        cxp[:NCL] = cx
        vxp = np.zeros((NVLp, VF), F32)
        vxp[:NVL] = vx
        m["cons_rows"] = cxp.reshape(NCLp // 128, 128, CF).transpose(1, 0, 2).copy()
        m["var_rows"] = vxp.reshape(NVLp // 128, 128, VF).transpose(1, 0, 2).copy()
        m["consT_aug"] = np.concatenate([cxp.T, np.ones((1, NCLp), F32)], 0).astype(BF16)
        m["varT_aug"] = np.concatenate([vxp.T, np.ones((1, NVLp), F32)], 0).astype(BF16)
        s1, oh1, ohT1 = p["conv1"].core_arrays(c)
        m["e1_src"], m["e1_oh"], m["e1_ohT"] = s1, oh1, ohT1
        s2, oh2, ohT2 = p["conv2"].core_arrays(c)
        m["e2_src"], m["e2_oh"], m["e2_ohT"] = s2, oh2, ohT2
        for k, v in w.items():
            m[k] = v
        core_inputs.append(m)
    p["core_inputs"] = core_inputs
    return p


# ---------------------------------------------------------------------------
# program builder
# ---------------------------------------------------------------------------


class B:
    """Builder context."""

    def __init__(self, p):
        self.p = p
        self.nc = bacc.Bacc("TRN2", target_bir_lowering=False, debug=False,
                            num_devices=NCORES)
        self.d = {}  # dram tensors

    def dram(self, name, shape, dtype, kind=None, addr_space=None):
        kw = {}
        if kind:
            kw["kind"] = kind
        if addr_space:
            kw["addr_space"] = addr_space
        t = self.nc.dram_tensor(name, list(shape), dtype, **kw)
        self.d[name] = t
        return t


MAGIC = 0x5F3759DF


def rsqrt_newton(nc, pool, src_ap, n, tag):
    """1/sqrt(src) on DVE only (2 Newton iters). src_ap [128, n] f32 > 0."""
    AL = mybir.AluOpType
    sh = pool.tile([128, n], dt.int32, tag=tag + "sh")
    nc.vector.tensor_scalar(sh[:], src_ap.bitcast(dt.int32), 1, None,
                            AL.arith_shift_right)
    y0 = pool.tile([128, n], dt.int32, tag=tag + "y0")
    nc.vector.tensor_scalar(y0[:], sh[:], -1, MAGIC, AL.mult, AL.add)
    cur = y0[:].bitcast(dt.float32)
    h = pool.tile([128, n], dt.float32, tag=tag + "h")
    nc.vector.tensor_scalar_mul(h[:], src_ap, 0.5)
    yy = pool.tile([128, n], dt.float32, tag=tag + "yy")
    for it in range(2):
        nc.vector.tensor_tensor(yy[:], cur, cur, AL.mult)
        nc.vector.tensor_tensor(yy[:], yy[:], h[:], AL.mult)
        nc.vector.tensor_scalar(yy[:], yy[:], -1.0, 1.5, AL.mult, AL.add)
        nxt = pool.tile([128, n], dt.float32, tag=tag + f"n{it}")
        nc.vector.tensor_tensor(nxt[:], cur, yy[:], AL.mult)
        cur = nxt[:]
    return cur


def ln_rows_stats(nc, pool, x_ap, nrows, nfeat):
    """LN stats for rows-major f32 [nrows, nfeat] -> (rstd_ap, nmr, mu). DVE only."""
    s1 = pool.tile([128, 1], dt.float32, tag="s1")
    nc.vector.reduce_sum(s1[:nrows], x_ap, axis=mybir.AxisListType.X)
    sq = pool.tile([128, nfeat], dt.float32, tag="sqscratch")
    s2 = pool.tile([128, 1], dt.float32, tag="s2")
    nc.vector.scalar_tensor_tensor(
        sq[:nrows], x_ap, 0.0, x_ap, mybir.AluOpType.add, mybir.AluOpType.mult,
        accum_out=s2[:nrows],
    )
    inv = 1.0 / nfeat
    musq = pool.tile([128, 1], dt.float32, tag="musq")
    nc.vector.scalar_tensor_tensor(
        musq[:nrows], s1[:nrows], inv * inv, s1[:nrows],
        mybir.AluOpType.mult, mybir.AluOpType.mult,
    )
    veps0 = pool.tile([128, 1], dt.float32, tag="veps0")
    nc.vector.tensor_scalar(veps0[:nrows], s2[:nrows], inv, EPS,
                            mybir.AluOpType.mult, mybir.AluOpType.add)
    veps = pool.tile([128, 1], dt.float32, tag="veps")
    nc.vector.tensor_tensor(veps[:nrows], veps0[:nrows], musq[:nrows],
                            mybir.AluOpType.subtract)
    rstd = rsqrt_newton(nc, pool, veps[:nrows], 1, "lnr")
    mu = pool.tile([128, 1], dt.float32, tag="mu")
    nc.vector.tensor_scalar_mul(mu[:nrows], s1[:nrows], inv)
    nmr = pool.tile([128, 1], dt.float32, tag="nmr")
    nc.vector.scalar_tensor_tensor(
        nmr[:nrows], mu[:nrows], -1.0, rstd,
        mybir.AluOpType.mult, mybir.AluOpType.mult,
    )
    return rstd, nmr, mu


def build_program(p):
    b = B(p)
    nc = b.nc
    w = p["weights"]
    NCL, NVL, NCLp, NVLp = p["NCL"], p["NVL"], p["NCLp"], p["NVLp"]
    CF, VF = p["CF"], p["VF"]
    NC, NV = p["NC"], p["NV"]
    NVLh = p["NVLh"]
    nact = p["nact"]

    # ---- dram declarations ----
    din = lambda n, s, t: b.dram(n, s, t, kind="ExternalInput")
    din("cons_rows", [128, NCLp // 128, CF], dt.float32)
    din("var_rows", [128, NVLp // 128, VF], dt.float32)
    din("consT_aug", [CF + 1, NCLp], dt.bfloat16)
    din("varT_aug", [VF + 1, NVLp], dt.bfloat16)
    c1p, c2p = p["conv1"], p["conv2"]
    din("e1_src", [128, c1p.etot // 16], dt.int16)
    din("e1_oh", [128, c1p.etot], dt.float8e4)
    din("e1_ohT", [128, c1p.etot], dt.float8e4)
    din("e2_src", [128, c2p.etot // 16], dt.int16)
    din("e2_oh", [128, c2p.etot], dt.float8e4)
    din("e2_ohT", [128, c2p.etot], dt.float8e4)
    for k, v in w.items():
        dtt = dt.bfloat16 if v.dtype == BF16 else (dt.int16 if v.dtype == np.int16 else dt.float32)
        din(k, list(v.shape), dtt)
    out_d = b.dram("out", [1, NVLh], dt.float32, kind="ExternalOutput")

    lp1_loc = b.dram("lp1_loc", [NVL, EMB], dt.bfloat16)
    lp1_full = b.dram("lp1_full", [NV, EMB], dt.bfloat16, addr_space="Shared")
    rp1_loc = b.dram("rp1_loc", [NCL, EMB], dt.bfloat16)
    lp2_loc = b.dram("lp2_loc", [NCL, EMB], dt.bfloat16)
    lp2_full = b.dram("lp2_full", [NC, EMB], dt.bfloat16, addr_space="Shared")
    rp2_loc = b.dram("rp2_loc", [NVL, EMB], dt.bfloat16)

    LR = mybir.ActivationFunctionType.Lrelu
    CP = mybir.ActivationFunctionType.Copy
    AL = mybir.AluOpType

    with tile.TileContext(nc) as tc:
        nc.gpsimd.load_library(library_config.mlp)
        with (
            tc.tile_pool(name="const", bufs=1) as cpool,
            tc.tile_pool(name="resident", bufs=1) as rpool,
            tc.tile_pool(name="work", bufs=3) as wpool,
            tc.tile_pool(name="tiny", bufs=4) as tpool,
            tc.tile_pool(name="gath", bufs=2) as gpool,
            tc.tile_pool(name="psA", bufs=2, space="PSUM") as psA,
            tc.tile_pool(name="psT", bufs=2, space="PSUM") as psT,
            tc.tile_pool(name="psagg", bufs=3, space="PSUM") as psagg,
            tc.tile_pool(name="psout", bufs=1, space="PSUM") as psout,
        ):
            # ---- load constants into SBUF ----
            cw = {}
            for k, v in w.items():
                dtt = dt.bfloat16 if v.dtype == BF16 else dt.float32
                t = cpool.tile(list(v.shape), dtt, tag=k)
                nc.sync.dma_start(t[:], b.d[k][:])
                cw[k] = t

            ident = cw["identity"]
            zero_col = cpool.tile([128, 1], dt.float32, tag="zero_col")
            nc.vector.memset(zero_col[:], 0.0)


            # residents
            c0T = rpool.tile([128, NCLp], dt.bfloat16, tag="c0T")
            v0T = rpool.tile([128, NVLp], dt.bfloat16, tag="v0T")
            c1T = rpool.tile([128, NCLp], dt.bfloat16, tag="c1T")
            v1T = rpool.tile([128, NVLh], dt.bfloat16, tag="v1T")
            nc.vector.memset(v1T[:], 0.0)

            def transpose_to(dst_ap, src_ap, n_p, n_f):
                """dst[:n_f, :n_p] = src[:n_p, :n_f].T via PE; dst bf16 SBUF."""
                ps = psT.tile([128, 128], dt.bfloat16, tag="psT")
                nc.tensor.transpose(ps[:n_f, :n_p], src_ap, ident[:n_p, :n_p])
                nc.scalar.copy(dst_ap, ps[:n_f, :n_p])

            # =========== stage A: embeddings (sharded rows) ===========
            def embed(pre, xT_aug_name, rows_name, nrows_p, nfeat, outT, extra):
                """Two-layer embed MLP. outT <- bf16 [128, nrows_p] transposed
                result. extra: list of (wname, biasname_or_None, dram_out,
                alsoT_or_None) projections computed from outT chunks."""
                nchunks = nrows_p // 128
                for ch in range(nchunks):
                    xTa = wpool.tile([nfeat + 1, 128], dt.bfloat16, tag="xTa")
                    nc.sync.dma_start(xTa[:], b.d[xT_aug_name][:, ch * 128 : (ch + 1) * 128])
                    xrows = wpool.tile([128, nfeat], dt.float32, tag="xrows")
                    nc.sync.dma_start(xrows[:], b.d[rows_name][:, ch, :])
                    rstd, nmr, mu = ln_rows_stats(nc, tpool, xrows[:], 128, nfeat)
                    ps = psA.tile([128, EMB], dt.float32, tag="ps")
                    nc.tensor.matmul(ps[:], xTa[:],
                                     cw[pre + "w1aug"][:], start=True, stop=True)
                    tmid = wpool.tile([128, EMB], dt.float32, tag="embmid")
                    nc.vector.scalar_tensor_tensor(
                        tmid[:], cw[pre + "r1"][:], mu[:], ps[:],
                        AL.mult, AL.subtract)
                    nrstd = tpool.tile([128, 1], dt.float32, tag="nrstd")
                    nc.vector.tensor_scalar_mul(nrstd[:], rstd, -1.0)
                    tmid2 = wpool.tile([128, EMB], dt.float32, tag="tmid2")
                    nc.vector.scalar_tensor_tensor(
                        tmid2[:], tmid[:], nrstd[:], cw[pre + "s1"][:], AL.mult, AL.add)
                    z1 = wpool.tile([128, EMB], dt.bfloat16, tag="z1")
                    nc.scalar.activation(z1[:], tmid2[:], LR, bias=zero_col[:], alpha=SLOPE)
                    z1T = wpool.tile([128, 128], dt.bfloat16, tag="z1T")
                    transpose_to(z1T[:], z1[:], 128, 128)
                    ps2 = psA.tile([128, EMB], dt.float32, tag="ps")
                    nc.tensor.matmul(ps2[:], z1T[:], cw[pre + "w2"][:], start=True, stop=True)
                    u = wpool.tile([128, EMB], dt.float32, tag="embu")
                    nc.vector.tensor_add(u[:], ps2[:], cw[pre + "b2"][:])
                    z2 = wpool.tile([128, EMB], dt.bfloat16, tag="z2")
                    nc.scalar.activation(z2[:], u[:], LR, bias=zero_col[:], alpha=SLOPE)
                    transpose_to(outT[:, ch * 128 : (ch + 1) * 128], z2[:], 128, 128)
                    # projections from outT chunk
                    for (wname, bname, dout, n_valid) in extra:
                        lo = ch * 128
                        nv = min(128, max(0, n_valid - lo))
                        if nv == 0:
                            continue
                        ps3 = psA.tile([128, EMB], dt.float32, tag="ps")
                        nc.tensor.matmul(ps3[:], outT[:, lo : lo + 128],
                                         cw[wname][:], start=True, stop=True)
                        ob = wpool.tile([128, EMB], dt.bfloat16, tag="projo")
                        if bname is not None:
                            ub = wpool.tile([128, EMB], dt.float32, tag="proju")
                            nc.vector.tensor_add(ub[:], ps3[:], cw[bname][:])
                            nc.scalar.copy(ob[:], ub[:])
                        else:
                            nc.scalar.copy(ob[:], ps3[:])
                        nc.sync.dma_start(b.d[dout][lo : lo + nv, :], ob[:nv, :])

            KSTAGE = os.environ.get("KSTAGE", "full")
            embed("ve_", "varT_aug", "var_rows", NVLp, VF, v0T,
                  [("vc_wl", "vc_bl", "lp1_loc", NVL), ("cv_wr", None, "rp2_loc", NVL)])
            # all-gather lp1 early (overlaps cons embed + conv prep)
            if KSTAGE != "A":
                nc.gpsimd.collective_compute(
                    "AllGather", AL.bypass, ins=[lp1_loc[:]], outs=[lp1_full[:]],
                    replica_groups=[list(range(NCORES))])
            embed("ce_", "consT_aug", "cons_rows", NCLp, CF, c0T,
                  [("vc_wr", None, "rp1_loc", NCL)])

            # =========== conv edge stage ===========
            def conv_edges(cv, pre, lp_dram, rp_dram, src_d, oh_d, ohT_d, acc,
                           n_valid):
                fl_triv = p[pre + "fl_trivial"]
                rp_tiles = {}

                def get_rp(blk):
                    if blk in rp_tiles:
                        return rp_tiles[blk]
                    rp_sb = wpool.tile([128, EMB], dt.bfloat16, tag="rpblk")
                    lo = blk * 128
                    nv = min(128, n_valid - lo)
                    if nv < 128:
                        nc.vector.memset(rp_sb[:], 0.0)
                    nc.sync.dma_start(rp_sb[:nv, :], rp_dram[lo : lo + nv, :])
                    rp_tiles[blk] = rp_sb
                    return rp_sb

                for stream in cv.streams:
                    rp_tiles.clear()
                    base_edge = stream["start_edge"]
                    ntiles = stream["ntiles"]
                    view_lo = HI_BASE if stream["u"] == 1 else 0
                    lp_view = lp_dram[view_lo:, :] if view_lo else lp_dram[:, :]
                    blk_of_tile = {}
                    t0 = 0
                    for (blk, tcnt) in stream["blocks"]:
                        for t in range(t0, t0 + tcnt):
                            blk_of_tile[t] = (blk, t == t0, t == t0 + tcnt - 1)
                        t0 += tcnt
                    cur_ps = [None]

                    tdone = 0
                    while tdone < ntiles:
                        tcn = min(CHUNK_TILES, ntiles - tdone)
                        e0 = base_edge + tdone * 128
                        ne = tcn * 128
                        sidx = gpool.tile([128, ne // 16], dt.int16, tag="sidx")
                        nc.sync.dma_start(sidx[:], src_d[:, e0 // 16 : (e0 + ne) // 16])
                        sbuf = gpool.tile([128, ne // 128, EMB], dt.bfloat16, tag="sgat")
                        nc.gpsimd.dma_gather(sbuf[:], lp_view, sidx[:], ne, ne, EMB,
                                             single_packet=False)
                        ohe = gpool.tile([128, ne], dt.float8e4, tag="ohe")
                        nc.sync.dma_start(ohe[:], oh_d[:, e0 : e0 + ne])
                        ohT = gpool.tile([128, ne], dt.float8e4, tag="ohT")
                        nc.sync.dma_start(ohT[:], ohT_d[:, e0 : e0 + ne])

                        xw_c = gpool.tile([128, tcn, EMB], dt.bfloat16, tag="xwc")
                        mv_c = tpool.tile([128, tcn, 2], dt.float32, tag="mvc")
                        st6 = tpool.tile([128, 6], dt.float32, tag="st6")

                        # pass A: rp broadcast, add, stats
                        for ti in range(tcn):
                            blk, _, _ = blk_of_tile[tdone + ti]
                            rp_sb = get_rp(blk)
                            rpe = psA.tile([128, EMB], dt.float32, tag="ps")
                            nc.tensor.matmul(rpe[:], ohT[:, ti * 128 : (ti + 1) * 128],
                                             rp_sb[:], start=True, stop=True)
                            nc.vector.tensor_add(xw_c[:, ti, :], sbuf[:, ti, :], rpe[:])
                            nc.vector.bn_stats(st6[:], xw_c[:, ti, :])
                            nc.vector.bn_aggr(mv_c[:, ti, :], st6[:])

                        # chunk stats finalize on DVE only
                        veps = tpool.tile([128, tcn], dt.float32, tag="vepsc")
                        nc.vector.tensor_scalar(veps[:], mv_c[:, :, 1], EPS, None, AL.add)
                        rstd_c = rsqrt_newton(nc, tpool, veps[:], tcn, "cvr")
                        nmr_c = tpool.tile([128, tcn], dt.float32, tag="nmrc")
                        nc.vector.scalar_tensor_tensor(
                            nmr_c[:], mv_c[:, :, 0], -1.0, rstd_c, AL.mult, AL.mult)
                        rstd_t = tpool.tile([128, tcn], dt.float32, tag="rstdc")
                        nc.vector.tensor_copy(rstd_t[:], rstd_c)

                        # pass B: apply + aggregate
                        for ti in range(tcn):
                            blk, isfirst, islast = blk_of_tile[tdone + ti]
                            act = wpool.tile([128, EMB + 1], dt.bfloat16, tag="act")
                            nc.vector.memset(act[:, EMB : EMB + 1], 1.0)
                            if fl_triv:
                                nc.scalar.activation(
                                    act[:, :EMB], xw_c[:, ti, :], LR,
                                    bias=nmr_c[:, ti : ti + 1],
                                    scale=rstd_t[:, ti : ti + 1], alpha=SLOPE)
                            else:
                                y1 = wpool.tile([128, EMB], dt.float32, tag="y1")
                                nc.vector.tensor_scalar(
                                    y1[:], xw_c[:, ti, :], mv_c[:, ti, 0:1],
                                    rstd_t[:, ti : ti + 1], AL.subtract, AL.mult)
                                y2 = wpool.tile([128, EMB], dt.float32, tag="y2")
                                nc.vector.scalar_tensor_tensor(
                                    y2[:], y1[:], 1.0, cw[pre + "flg"][:], AL.mult, AL.mult)
                                y3 = wpool.tile([128, EMB], dt.float32, tag="y3")
                                nc.vector.tensor_add(y3[:], y2[:], cw[pre + "flb"][:])
                                nc.scalar.activation(act[:, :EMB], y3[:], LR,
                                                     bias=zero_col[:], alpha=SLOPE)
                            if cur_ps[0] is None:
                                psb_new = psagg.tile([128, EMB + 1], dt.float32, tag="agg")
                                cur_ps[0] = psb_new
                            psb = cur_ps[0]
                            nc.tensor.matmul(
                                psb[:], ohe[:, ti * 128 : (ti + 1) * 128], act[:],
                                start=isfirst, stop=islast)
                            if islast:
                                nc.vector.tensor_add(acc[:, blk, :], acc[:, blk, :], psb[:])
                                cur_ps[0] = None
                        tdone += tcn

            # =========== post-conv: mean -> wf -> LN -> MLP ===========
            def conv_post(cv, pre, acc, rightT, outT, lpout_name, lpout_w, lpout_b,
                          n_valid):
                nblocks = cv.nblocks
                for blk in range(nblocks):
                    lo = blk * 128
                    nv = min(128, n_valid - lo)
                    cnt = tpool.tile([128, 1], dt.float32, tag="cnt")
                    nc.vector.tensor_scalar_max(cnt[:], acc[:, blk, EMB : EMB + 1], 1.0)
                    rec = tpool.tile([128, 1], dt.float32, tag="rec")
                    nc.vector.reciprocal(rec[:], cnt[:])
                    mean = wpool.tile([128, EMB], dt.bfloat16, tag="mean")
                    nc.vector.tensor_scalar_mul(mean[:], acc[:, blk, :EMB], rec[:])
                    meanT = wpool.tile([128, 128], dt.bfloat16, tag="meanT")
                    transpose_to(meanT[:], mean[:], 128, 128)
                    ps = psA.tile([128, EMB], dt.float32, tag="ps")
                    nc.tensor.matmul(ps[:], meanT[:], cw[pre + "wf"][:], start=True, stop=True)
                    ind = tpool.tile([128, 1], dt.float32, tag="ind")
                    nc.vector.tensor_scalar_min(ind[:], acc[:, blk, EMB : EMB + 1], 1.0)
                    u = wpool.tile([128, EMB], dt.float32, tag="pcu")
                    nc.vector.scalar_tensor_tensor(
                        u[:], cw[pre + "bf"][:], ind[:], ps[:], AL.mult, AL.add)
                    # LN (plg/plb folded into wo1a/bo1)
                    rstd, nmr, mu = ln_rows_stats(nc, tpool, u[:], 128, EMB)
                    lnv = wpool.tile([128, EMB], dt.bfloat16, tag="lnv")
                    nc.vector.tensor_scalar(
                        lnv[:], u[:], mu[:], rstd, AL.subtract, AL.mult)
                    lnT = wpool.tile([128, 128], dt.bfloat16, tag="lnT")
                    transpose_to(lnT[:], lnv[:], 128, 128)
                    ps2 = psA.tile([128, EMB], dt.float32, tag="ps")
                    nc.tensor.matmul(ps2[:], lnT[:], cw[pre + "wo1a"][:], start=True, stop=False)
                    nc.tensor.matmul(ps2[:], rightT[:, lo : lo + 128], cw[pre + "wo1b"][:],
                                     start=False, stop=True)
                    u2 = wpool.tile([128, EMB], dt.float32, tag="pcu2")
                    nc.vector.tensor_add(u2[:], ps2[:], cw[pre + "bo1"][:])
                    tml = wpool.tile([128, EMB], dt.bfloat16, tag="tml")
                    nc.scalar.activation(tml[:], u2[:], LR, bias=zero_col[:], alpha=SLOPE)
                    tT = wpool.tile([128, 128], dt.bfloat16, tag="tT")
                    transpose_to(tT[:], tml[:], 128, 128)
                    ps3 = psA.tile([128, EMB], dt.float32, tag="ps")
                    nc.tensor.matmul(ps3[:], tT[:], cw[pre + "wo2"][:], start=True, stop=True)
                    u3 = wpool.tile([128, EMB], dt.float32, tag="pcu3")
                    nc.vector.tensor_add(u3[:], ps3[:], cw[pre + "bo2"][:])
                    res = wpool.tile([128, EMB], dt.bfloat16, tag="res")
                    nc.scalar.copy(res[:], u3[:])
                    transpose_to(outT[:, lo : lo + 128], res[:], 128, 128)
                    if lpout_name is not None and nv > 0:
                        ps4 = psA.tile([128, EMB], dt.float32, tag="ps")
                        nc.tensor.matmul(ps4[:], outT[:, lo : lo + 128], cw[lpout_w][:],
                                         start=True, stop=True)
                        ub = wpool.tile([128, EMB], dt.float32, tag="pc4u")
                        nc.vector.tensor_add(ub[:], ps4[:], cw[lpout_b][:])
                        ob = wpool.tile([128, EMB], dt.bfloat16, tag="pc4o")
                        nc.scalar.copy(ob[:], ub[:])
                        nc.sync.dma_start(b.d[lpout_name][lo : lo + nv, :], ob[:nv, :])

            # conv1
            acc1 = rpool.tile([128, c1p.nblocks, EMB + 1], dt.float32, tag="acc1")
            nc.vector.memset(acc1[:], 0.0)
            if KSTAGE not in ("A", "AG1"):
                _lp1src = rp1_loc if KSTAGE == "C1local" else lp1_full
                conv_edges(c1p, "vc_", _lp1src, rp1_loc, b.d["e1_src"],
                           b.d["e1_oh"], b.d["e1_ohT"], acc1, NCL)
            if KSTAGE not in ("A", "AG1", "C1"):
                conv_post(c1p, "vc_", acc1, c0T, c1T, "lp2_loc", "cv_wl", "cv_bl", NCL)
                nc.gpsimd.collective_compute(
                    "AllGather", AL.bypass, ins=[lp2_loc[:]], outs=[lp2_full[:]],
                    replica_groups=[list(range(NCORES))])
            # conv2
            acc2 = rpool.tile([128, c2p.nblocks, EMB + 1], dt.float32, tag="acc2")
            nc.vector.memset(acc2[:], 0.0)
            if KSTAGE not in ("A", "AG1", "C1", "P1"):
                conv_edges(c2p, "cv_", lp2_full, rp2_loc, b.d["e2_src"],
                           b.d["e2_oh"], b.d["e2_ohT"], acc2, NVL)
                conv_post(c2p, "cv_", acc2, v0T, v1T, None, None, None, NVL)

            # =========== heads ===========
            outrow = rpool.tile([1, NVLh], dt.float32, tag="outrow")
            if KSTAGE != "full":
                nc.vector.memset(outrow[:], 0.0)
            elif nact == 0:
                nc.vector.memset(outrow[:], 0.0)
            elif True:
                nch = NVLh // 512
                for j in range(nch):
                    pso = psout.tile([1, 512], dt.float32, tag="pso")
                    for hi in range(nact):
                        ps = psA.tile([128, 512], dt.float32, tag="ps")
                        nc.tensor.matmul(ps[:], cw["hw1"][:, hi, :],
                                         v1T[:, j * 512 : (j + 1) * 512],
                                         start=True, stop=True)
                        hh = wpool.tile([128, 512], dt.bfloat16, tag="hh")
                        nc.scalar.activation(hh[:], ps[:], LR,
                                             bias=cw["hb1"][:, hi : hi + 1],
                                             scale=1.0, alpha=SLOPE)
                        nc.tensor.matmul(pso[:], cw["hw2"][:, hi : hi + 1], hh[:],
                                         start=(hi == 0), stop=(hi == nact - 1))
                    nc.vector.tensor_scalar(
                        outrow[:, j * 512 : (j + 1) * 512], pso[:],
                        p["out_scale"], p["out_add"], AL.mult, AL.add)
            nc.sync.dma_start(out_d[:], outrow[:])

    nc.compile()
    return b


_CACHE = {}


def kernel(**inputs):
    key = tuple(sorted((k, tuple(np.asarray(v).shape)) for k, v in inputs.items()))
    p = host_prep(inputs)
    ck = (key, p["nact"], p["conv1"].etot, p["conv2"].etot,
          p["vc_fl_trivial"], p["cv_fl_trivial"])
    if ck in _CACHE:
        b = _CACHE[ck]
    else:
        b = build_program(p)
        _CACHE[ck] = b
    in_maps = [dict(p["core_inputs"][c]) for c in range(NCORES)]
    res = run_bass_kernel_spmd(b.nc, in_maps, core_ids=list(range(NCORES)))
    NVL = p["NVL"]
    out = np.concatenate([res.results[c]["out"][0, :NVL] for c in range(NCORES)])
    return out.astype(np.float32)



# revision 14
# speedup vs baseline: 1.1508x; 1.1508x over previous
"""Trainium2 Bass kernel for BipartiteGCN (8 NeuronCores, SPMD) — v2.

Strategy:
 - Node rows sharded 8 ways; edges sharded by DESTINATION range, grouped by
   (dst_block, src_bucket) segments with src-sorted order inside.
 - Edge loop (edge-major [e, f] tiles of 128 edges):
   dma_gather lp[src] rows (bf16), one-hot matmul broadcasts rp[dst] into
   PSUM, one DVE add per 4 tiles, grouped bn_stats, ScalarE fused
   (x-mu)*rstd + LeakyReLU, acc^T matmul (feature-major segment sum,
   4 dst-blocks per PSUM bank).
 - Scatter-mean denominators precomputed on host (no ones column).
 - Post-conv + embed MLPs run feature-major: batched N=512 matmuls with
   stationary weights, LN via colsum matmuls + row math, rank-1 bias /
   mean-correction matmuls, biases as per-partition columns. No PE
   transposes anywhere.
 - lp tables all-gathered (bf16); heads interleaved with conv2 post.
"""

import os
import sys

for _p in ("/opt/trn_rl_repo",):
    if _p not in sys.path:
        sys.path.insert(0, _p)

import numpy as np
import ml_dtypes

import concourse.bass as bass
import concourse.bacc as bacc
import concourse.mybir as mybir
from concourse import tile, library_config
from concourse.bass_utils import run_bass_kernel_spmd

BF16 = ml_dtypes.bfloat16
F32 = np.float32
NCORES = 8
EMB = 128
CHUNK_TILES = 16
HI_BASE = 32768
EPS = 1e-5
SLOPE = 0.01

dt = mybir.dt
AL = mybir.AluOpType
LR = mybir.ActivationFunctionType.Lrelu
SQ = mybir.ActivationFunctionType.Sqrt


def _wrap_idx(idx_i16):
    n = idx_i16.shape[0]
    assert n % 16 == 0
    w = idx_i16.reshape(n // 16, 16).T
    return np.tile(w, (8, 1)).copy()


def _col(v):
    return np.asarray(v, F32)[:, None].copy()


def _bcast_row(v, rows=128):
    return np.broadcast_to(np.asarray(v, F32)[None, :], (rows, v.shape[0])).copy()


class ConvPrep:
    """Per-conv edge data: segments of (dst_block, src_bucket), src-sorted."""

    def __init__(self, dst, src, n_dst, n_src, dst_per_core):
        self.n_dst_local = dst_per_core
        self.nblocks = -(-dst_per_core // 128)
        self.two_buckets = n_src > HI_BASE
        nb = self.nblocks
        nu = 2 if self.two_buckets else 1

        core = dst // dst_per_core
        dloc_all = dst - core * dst_per_core
        block_all = dloc_all // 128

        self.counts = np.zeros((NCORES, dst_per_core), np.int64)
        for c in range(NCORES):
            m = core == c
            self.counts[c] = np.bincount(dloc_all[m], minlength=dst_per_core)

        per = [[[None] * nu for _ in range(nb)] for _ in range(NCORES)]
        for c in range(NCORES):
            m = core == c
            d_c = dloc_all[m]
            s_c = src[m]
            b_c = block_all[m]
            u_c = (s_c >= HI_BASE).astype(np.int8) if self.two_buckets else np.zeros(
                len(s_c), np.int8
            )
            for bq in range(nb):
                for u in range(nu):
                    mu = (b_c == bq) & (u_c == u)
                    order = np.argsort(s_c[mu], kind="stable")
                    per[c][bq][u] = (s_c[mu][order], d_c[mu][order])

        self.ntiles = np.zeros((nb, nu), np.int64)
        for bq in range(nb):
            for u in range(nu):
                mx = max(len(per[c][bq][u][0]) for c in range(NCORES))
                self.ntiles[bq, u] = -(-mx // 128) if mx > 0 else 0

        etot = int(self.ntiles.sum()) * 128
        self.etot = etot

        self.src_idx = np.zeros((NCORES, etot), np.int16)
        self.dstrel = np.full((NCORES, etot), -1.0, F32)
        self.segments = []
        off = 0
        for bq in range(nb):
            for u in range(nu):
                g = int(self.ntiles[bq, u])
                if g == 0:
                    continue
                self.segments.append(
                    {"u": u, "blk": bq, "ntiles": g, "start_edge": off})
                for c in range(NCORES):
                    sb, db = per[c][bq][u]
                    n = len(sb)
                    s_adj = sb - (HI_BASE if u == 1 else 0)
                    self.src_idx[c, off : off + n] = s_adj.astype(np.int16)
                    self.dstrel[c, off : off + n] = (db - 128 * bq).astype(F32)
                off += g * 128
        assert off == etot

    def core_arrays(self, c):
        dr = self.dstrel[c]
        i = np.nonzero(dr >= 0)[0]
        lane = i % 128
        tb = (i // 128) * 128
        d = dr[i].astype(np.int64)
        oh = np.zeros((128, self.etot), ml_dtypes.float8_e4m3)
        oh[lane, tb + d] = 1.0
        ohT = np.zeros((128, self.etot), ml_dtypes.float8_e4m3)
        ohT[d, tb + lane] = 1.0
        return _wrap_idx(self.src_idx[c]), oh, ohT

    def count_arrays(self, c, n_pad):
        cnt = self.counts[c].astype(F32)
        rcnt = 1.0 / np.maximum(cnt, 1.0)
        ind = (cnt > 0).astype(F32)
        rcnt_p = np.zeros(n_pad, F32)
        rcnt_p[: len(rcnt)] = rcnt
        ind_p = np.zeros(n_pad, F32)
        ind_p[: len(ind)] = ind
        return (
            rcnt_p[None, :].astype(BF16).copy(),
            ind_p[None, :].astype(BF16).copy(),
        )


def host_prep(inputs):
    p = {}
    cons_x = np.asarray(inputs["cons_x"], F32)
    var_x = np.asarray(inputs["var_x"], F32)
    edge_cons = np.asarray(inputs["edge_cons"]).astype(np.int64)
    edge_var = np.asarray(inputs["edge_var"]).astype(np.int64)
    head_mask = np.asarray(inputs["head_mask"]).astype(bool)

    NC, CF = cons_x.shape
    NV, VF = var_x.shape
    assert NC % NCORES == 0 and NV % NCORES == 0
    NCL, NVL = NC // NCORES, NV // NCORES
    p.update(NC=NC, NV=NV, CF=CF, VF=VF, NCL=NCL, NVL=NVL)

    p["conv1"] = ConvPrep(edge_cons, edge_var, NC, NV, NCL)
    p["conv2"] = ConvPrep(edge_var, edge_cons, NV, NC, NVL)

    NCLp = -(-NCL // 128) * 128
    NVLp = -(-NVL // 128) * 128
    NCLg = -(-NCLp // 512) * 512
    NVLg = -(-NVLp // 512) * 512
    p.update(NCLp=NCLp, NVLp=NVLp, NCLg=NCLg, NVLg=NVLg)

    w = {}

    def embed_w(prefix, g, b, w1, b1, w2, b2, feat):
        g = np.asarray(g, F32)
        b = np.asarray(b, F32)
        w1 = np.asarray(w1, F32)
        w1g = g[:, None] * w1
        aug = np.concatenate([w1g, np.zeros((1, w1.shape[1]), F32)], 0)
        w[prefix + "w1aug"] = aug.astype(BF16)
        w[prefix + "negw1bar"] = (-w1g.sum(0))[None, :].astype(BF16)
        w[prefix + "s1"] = _col(b @ w1 + np.asarray(b1, F32))
        w[prefix + "w2"] = np.asarray(w2, F32).astype(BF16)
        w[prefix + "b2"] = _col(np.asarray(b2, F32))
        mv = np.zeros((feat + 1, 1), F32)
        mv[:feat, 0] = 1.0 / feat
        w[prefix + "meanvec"] = mv.astype(BF16)

    embed_w("ce_", inputs["ce_ln_g"], inputs["ce_ln_b"], inputs["ce_w1"],
            inputs["ce_b1"], inputs["ce_w2"], inputs["ce_b2"], CF)
    embed_w("ve_", inputs["ve_ln_g"], inputs["ve_ln_b"], inputs["ve_w1"],
            inputs["ve_b1"], inputs["ve_w2"], inputs["ve_b2"], VF)

    for pre in ("vc_", "cv_"):
        wl = np.asarray(inputs[pre + "wl"], F32)
        w[pre + "wl"] = wl.astype(BF16)
        w[pre + "bl_row"] = _bcast_row(np.asarray(inputs[pre + "bl"], F32))
        w[pre + "wr"] = np.asarray(inputs[pre + "wr"], F32).astype(BF16)
        wf = np.asarray(inputs[pre + "wf"], F32)
        flg = np.asarray(inputs[pre + "flg"], F32)
        flb = np.asarray(inputs[pre + "flb"], F32)
        p[pre + "fl_trivial"] = bool(np.all(flg == 1.0) and np.all(flb == 0.0))
        w[pre + "wf"] = wf.astype(BF16)
        w[pre + "wfbar"] = (wf.sum(1) / EMB)[:, None].astype(BF16)
        bf = np.asarray(inputs[pre + "bf"], F32)
        w[pre + "bf_row"] = bf[None, :].astype(BF16)
        w[pre + "bfbar1"] = np.full((1, 1), bf.sum() / EMB, BF16)
        wo1 = np.asarray(inputs[pre + "wo1"], F32)
        plg = np.asarray(inputs[pre + "plg"], F32)
        plb = np.asarray(inputs[pre + "plb"], F32)
        wo1a = plg[:, None] * wo1[:EMB]
        w[pre + "wo1a"] = wo1a.astype(BF16)
        w[pre + "wo1abar"] = wo1a.sum(0)[None, :].astype(BF16)
        w[pre + "wo1b"] = wo1[EMB:].astype(BF16)
        w[pre + "bo1"] = _col(np.asarray(inputs[pre + "bo1"], F32) + plb @ wo1[:EMB])
        w[pre + "wo2"] = np.asarray(inputs[pre + "wo2"], F32).astype(BF16)
        w[pre + "bo2"] = _col(np.asarray(inputs[pre + "bo2"], F32))

    active = np.nonzero(head_mask)[0]
    nact = int(len(active))
    p["nact"] = nact
    denom = max(float(head_mask.sum()), 1.0)
    hb2 = np.asarray(inputs["hb2"], F32)
    p["out_scale"] = 1.0 / denom
    p["out_add"] = float(hb2[active].sum() / denom)
    if nact > 0:
        hw1 = np.asarray(inputs["hw1"], F32)[active]
        w["hw1"] = hw1.transpose(1, 0, 2).astype(BF16).copy()
        w["hb1"] = np.asarray(inputs["hb1"], F32)[active].T.copy()
        w["hw2"] = np.asarray(inputs["hw2"], F32)[active].T.astype(BF16).copy()

    w["ones_row"] = np.ones((1, 128), BF16)
    w["invemb_col"] = np.full((128, 1), 1.0 / EMB, BF16)
    p["weights"] = w

    core_inputs = []
    for c in range(NCORES):
        m = {}
        cx = cons_x[c * NCL : (c + 1) * NCL]
        vx = var_x[c * NVL : (c + 1) * NVL]
        cxp = np.zeros((NCLp, CF), F32)
        cxp[:NCL] = cx
        vxp = np.zeros((NVLp, VF), F32)
        vxp[:NVL] = vx
        m["consT_aug"] = np.concatenate([cxp.T, np.ones((1, NCLp), F32)], 0).astype(BF16)
        m["varT_aug"] = np.concatenate([vxp.T, np.ones((1, NVLp), F32)], 0).astype(BF16)
        s1, oh1, ohT1 = p["conv1"].core_arrays(c)
        m["e1_src"], m["e1_oh"], m["e1_ohT"] = s1, oh1, ohT1
        s2, oh2, ohT2 = p["conv2"].core_arrays(c)
        m["e2_src"], m["e2_oh"], m["e2_ohT"] = s2, oh2, ohT2
        m["rcnt1"], m["ind1"] = p["conv1"].count_arrays(c, NCLg)
        m["rcnt2"], m["ind2"] = p["conv2"].count_arrays(c, NVLg)
        for k, v in w.items():
            m[k] = v
        core_inputs.append(m)
    p["core_inputs"] = core_inputs
    return p


# ---------------------------------------------------------------------------


class B:
    def __init__(self, p):
        self.p = p
        self.nc = bacc.Bacc("TRN2", target_bir_lowering=False, debug=False,
                            num_devices=NCORES)
        self.d = {}

    def dram(self, name, shape, dtype, kind=None, addr_space=None):
        kw = {}
        if kind:
            kw["kind"] = kind
        if addr_space:
            kw["addr_space"] = addr_space
        t = self.nc.dram_tensor(name, list(shape), dtype, **kw)
        self.d[name] = t
        return t


def build_program(p):
    b = B(p)
    nc = b.nc
    w = p["weights"]
    NCL, NVL, NCLp, NVLp = p["NCL"], p["NVL"], p["NCLp"], p["NVLp"]
    NCLg, NVLg = p["NCLg"], p["NVLg"]
    CF, VF = p["CF"], p["VF"]
    NC, NV = p["NC"], p["NV"]
    nact = p["nact"]

    din = lambda n, s, t: b.dram(n, s, t, kind="ExternalInput")
    din("consT_aug", [CF + 1, NCLp], dt.bfloat16)
    din("varT_aug", [VF + 1, NVLp], dt.bfloat16)
    c1p, c2p = p["conv1"], p["conv2"]
    din("e1_src", [128, c1p.etot // 16], dt.int16)
    din("e1_oh", [128, c1p.etot], dt.float8e4)
    din("e1_ohT", [128, c1p.etot], dt.float8e4)
    din("e2_src", [128, c2p.etot // 16], dt.int16)
    din("e2_oh", [128, c2p.etot], dt.float8e4)
    din("e2_ohT", [128, c2p.etot], dt.float8e4)
    din("rcnt1", [1, NCLg], dt.bfloat16)
    din("ind1", [1, NCLg], dt.bfloat16)
    din("rcnt2", [1, NVLg], dt.bfloat16)
    din("ind2", [1, NVLg], dt.bfloat16)
    for k, v in w.items():
        din(k, list(v.shape), dt.bfloat16 if v.dtype == BF16 else dt.float32)
    out_d = b.dram("out", [1, NVLg], dt.float32, kind="ExternalOutput")

    lp1_loc = b.dram("lp1_loc", [NVL, EMB], dt.bfloat16)
    lp1_full = b.dram("lp1_full", [NV, EMB], dt.bfloat16, addr_space="Shared")
    rp1_loc = b.dram("rp1_loc", [NCL, EMB], dt.bfloat16)
    lp2_loc = b.dram("lp2_loc", [NCL, EMB], dt.bfloat16)
    lp2_full = b.dram("lp2_full", [NC, EMB], dt.bfloat16, addr_space="Shared")
    rp2_loc = b.dram("rp2_loc", [NVL, EMB], dt.bfloat16)

    KSTAGE = os.environ.get("KSTAGE", "full")

    with tile.TileContext(nc) as tc:
        nc.gpsimd.load_library(library_config.mlp)
        with (
            tc.tile_pool(name="const", bufs=1) as cpool,
            tc.tile_pool(name="resident", bufs=1) as rpool,
            tc.tile_pool(name="work", bufs=2) as wpool,
            tc.tile_pool(name="tiny", bufs=2) as tpool,
            tc.tile_pool(name="gath", bufs=2) as gpool,
            tc.tile_pool(name="psx", bufs=2, space="PSUM") as psx_pool,
            tc.tile_pool(name="agg", bufs=2, space="PSUM") as agg_pool,
            tc.tile_pool(name="mm", bufs=2, space="PSUM") as mm_pool,
            tc.tile_pool(name="bc", bufs=2, space="PSUM") as bc_pool,
        ):
            cw = {}
            for k, v in w.items():
                dtt = dt.bfloat16 if v.dtype == BF16 else dt.float32
                t = cpool.tile(list(v.shape), dtt, tag=k)
                nc.sync.dma_start(t[:], b.d[k][:])
                cw[k] = t
            for k, shp in (("rcnt1", [1, NCLg]), ("ind1", [1, NCLg]),
                           ("rcnt2", [1, NVLg]), ("ind2", [1, NVLg])):
                t = cpool.tile(shp, dt.bfloat16, tag=k)
                nc.sync.dma_start(t[:], b.d[k][:])
                cw[k] = t

            c0T = rpool.tile([128, NCLg], dt.bfloat16, tag="c0T")
            v0T = rpool.tile([128, NVLg], dt.bfloat16, tag="v0T")
            c1T = rpool.tile([128, NCLg], dt.bfloat16, tag="c1T")
            v1T = rpool.tile([128, NVLg], dt.bfloat16, tag="v1T")
            nc.vector.memset(c0T[:], 0.0)
            nc.vector.memset(v0T[:], 0.0)
            outrow = rpool.tile([1, NVLg], dt.float32, tag="outrow")

            def rowmath_rstd(mu_row, ssq_row):
                """mu/ssq [1,512] f32 rows -> (rstd bf16, -mu*rstd bf16,
                mu bf16) rows [1,512]."""
                nmusq = tpool.tile([1, 512], dt.float32, tag="rm_nmusq")
                nc.vector.scalar_tensor_tensor(
                    nmusq[:], mu_row, -1.0, mu_row, AL.mult, AL.mult)
                veps = tpool.tile([1, 512], dt.float32, tag="rm_veps")
                nc.vector.scalar_tensor_tensor(
                    veps[:], ssq_row, EPS, nmusq[:], AL.add, AL.add)
                sd = tpool.tile([1, 512], dt.float32, tag="rm_sd")
                nc.scalar.activation(sd[:], veps[:], SQ)
                rstd = tpool.tile([1, 512], dt.float32, tag="rm_rstd")
                nc.vector.reciprocal(rstd[:], sd[:])
                rstd_bf = tpool.tile([1, 512], dt.bfloat16, tag="rm_rstdbf")
                nc.vector.tensor_copy(rstd_bf[:], rstd[:])
                nmur = tpool.tile([1, 512], dt.bfloat16, tag="rm_nmur")
                nc.vector.scalar_tensor_tensor(
                    nmur[:], mu_row, -1.0, rstd[:], AL.mult, AL.mult)
                mu_bf = tpool.tile([1, 512], dt.bfloat16, tag="rm_mubf")
                nc.vector.tensor_copy(mu_bf[:], mu_row)
                return rstd_bf, nmur, mu_bf

            # =========== embeds (feature-major, 512-col groups) ===========
            def embed(pre, xT_name, nfeat, ncols, ncols_g, outT, projs):
                xT = rpool.tile([nfeat + 1, ncols], dt.bfloat16, tag=pre + "xT")
                nc.sync.dma_start(xT[:], b.d[xT_name][:])
                for gi in range(ncols_g // 512):
                    cn = min(512, ncols - gi * 512)
                    if cn <= 0:
                        break
                    sl = slice(gi * 512, gi * 512 + cn)
                    xsq = wpool.tile([nfeat + 1, 512], dt.bfloat16, tag="exsq")
                    nc.vector.tensor_tensor(xsq[:, :cn], xT[:, sl], xT[:, sl],
                                            AL.mult)
                    pst = bc_pool.tile([128, 512], dt.float32, tag="bc")
                    nc.tensor.matmul(pst[0:1, :cn], cw[pre + "meanvec"][:],
                                     xT[:, sl], start=True, stop=True)
                    pst2 = bc_pool.tile([128, 512], dt.float32, tag="bc")
                    nc.tensor.matmul(pst2[0:1, :cn], cw[pre + "meanvec"][:],
                                     xsq[:, :cn], start=True, stop=True)
                    strow = tpool.tile([1, 512], dt.float32, tag="strow")
                    nc.vector.tensor_copy(strow[0:1, :cn], pst[0:1, :cn])
                    strow2 = tpool.tile([1, 512], dt.float32, tag="strow2")
                    nc.vector.tensor_copy(strow2[0:1, :cn], pst2[0:1, :cn])
                    rstd_bf, _, mu_bf = rowmath_rstd(strow[0:1, :], strow2[0:1, :])
                    psA = mm_pool.tile([128, 512], dt.float32, tag="mm")
                    nc.tensor.matmul(psA[:, :cn], cw[pre + "w1aug"][:], xT[:, sl],
                                     start=True, stop=False)
                    nc.tensor.matmul(psA[:, :cn], cw[pre + "negw1bar"][:],
                                     mu_bf[:, :cn], start=False, stop=True)
                    rb = bc_pool.tile([128, 512], dt.float32, tag="bc")
                    nc.tensor.matmul(rb[:, :cn], cw["ones_row"][:],
                                     rstd_bf[:, :cn], start=True, stop=True)
                    psA_sb = wpool.tile([128, 512], dt.bfloat16, tag="epsAsb")
                    nc.vector.tensor_copy(psA_sb[:, :cn], psA[:, :cn])
                    tmid = wpool.tile([128, 512], dt.bfloat16, tag="etmid")
                    nc.vector.tensor_tensor(tmid[:, :cn], psA_sb[:, :cn],
                                            rb[:, :cn], AL.mult)
                    z1 = wpool.tile([128, 512], dt.bfloat16, tag="ez1")
                    nc.scalar.activation(z1[:, :cn], tmid[:, :cn], LR,
                                         bias=cw[pre + "s1"][:], alpha=SLOPE)
                    psB = mm_pool.tile([128, 512], dt.float32, tag="mm")
                    nc.tensor.matmul(psB[:, :cn], cw[pre + "w2"][:], z1[:, :cn],
                                     start=True, stop=True)
                    nc.scalar.activation(outT[:, sl], psB[:, :cn], LR,
                                         bias=cw[pre + "b2"][:], alpha=SLOPE)
                    for (wname, brow, dout, n_valid) in projs:
                        for bi in range(4):
                            lo = gi * 512 + bi * 128
                            nv = min(128, max(0, n_valid - lo))
                            if nv == 0:
                                continue
                            psP = mm_pool.tile([128, 512], dt.float32, tag="mm")
                            nc.tensor.matmul(psP[:, :128], outT[:, lo : lo + 128],
                                             cw[wname][:], start=True, stop=True)
                            ob = wpool.tile([128, EMB], dt.bfloat16, tag="eob")
                            if brow is not None:
                                nc.vector.tensor_tensor(ob[:], psP[:, :128],
                                                        cw[brow][:], AL.add)
                            else:
                                nc.vector.tensor_copy(ob[:], psP[:, :128])
                            nc.sync.dma_start(b.d[dout][lo : lo + nv, :], ob[:nv, :])

            embed("ve_", "varT_aug", VF, NVLp, NVLg, v0T,
                  [("vc_wl", "vc_bl_row", "lp1_loc", NVL),
                   ("cv_wr", None, "rp2_loc", NVL)])
            if KSTAGE != "A":
                nc.gpsimd.collective_compute(
                    "AllGather", AL.bypass, ins=[lp1_loc[:]], outs=[lp1_full[:]],
                    replica_groups=[list(range(NCORES))])
            embed("ce_", "consT_aug", CF, NCLp, NCLg, c0T,
                  [("vc_wr", None, "rp1_loc", NCL)])

            # =========== conv (edges + interleaved feature-major post) =====
            def conv(cv, pre, lp_dram, rp_dram, src_d, oh_d, ohT_d, rightT, outT,
                     rcnt_name, ind_name, projs, n_valid, post_cb=None):
                nblocks = cv.nblocks
                ngroups = -(-nblocks // 4)
                grp_ps = [None] * ngroups
                grp_done = [0] * ngroups

                def post_group(g):
                    lo = g * 512
                    sl = slice(lo, lo + 512)
                    psG = grp_ps[g]
                    mean = wpool.tile([128, 512], dt.bfloat16, tag="pmean")
                    if psG is None:
                        nc.vector.memset(mean[:], 0.0)
                    else:
                        rcb = bc_pool.tile([128, 512], dt.float32, tag="bc")
                        nc.tensor.matmul(rcb[:], cw["ones_row"][:],
                                         cw[rcnt_name][:, sl], start=True,
                                         stop=True)
                        acc_sb = wpool.tile([128, 512], dt.bfloat16, tag="paccsb")
                        nc.vector.tensor_copy(acc_sb[:], psG[:])
                        grp_ps[g] = None
                        nc.vector.tensor_tensor(mean[:], acc_sb[:], rcb[:],
                                                AL.mult)
                    psU = mm_pool.tile([128, 512], dt.float32, tag="mm")
                    nc.tensor.matmul(psU[:], cw[pre + "wf"][:], mean[:],
                                     start=True, stop=False)
                    nc.tensor.matmul(psU[:], cw[pre + "bf_row"][:],
                                     cw[ind_name][:, sl], start=False, stop=True)
                    pst = bc_pool.tile([128, 512], dt.float32, tag="bc")
                    nc.tensor.matmul(pst[0:1, :], cw[pre + "wfbar"][:], mean[:],
                                     start=True, stop=False)
                    nc.tensor.matmul(pst[0:1, :], cw[pre + "bfbar1"][:],
                                     cw[ind_name][:, sl], start=False, stop=True)
                    u_sb = wpool.tile([128, 512], dt.bfloat16, tag="pusb")
                    nc.vector.tensor_copy(u_sb[:], psU[:])
                    usq = wpool.tile([128, 512], dt.bfloat16, tag="pmean")
                    nc.vector.tensor_tensor(usq[:], u_sb[:], u_sb[:], AL.mult)
                    pst2 = bc_pool.tile([128, 512], dt.float32, tag="bc")
                    nc.tensor.matmul(pst2[0:1, :], cw["invemb_col"][:], usq[:],
                                     start=True, stop=True)
                    strow = tpool.tile([1, 512], dt.float32, tag="strow")
                    nc.vector.tensor_copy(strow[0:1, :], pst[0:1, :])
                    strow2 = tpool.tile([1, 512], dt.float32, tag="strow2")
                    nc.vector.tensor_copy(strow2[0:1, :], pst2[0:1, :])
                    rstd_bf, nmur, _ = rowmath_rstd(strow[0:1, :], strow2[0:1, :])
                    rb = bc_pool.tile([128, 512], dt.float32, tag="bc")
                    nc.tensor.matmul(rb[:], cw["ones_row"][:], rstd_bf[:],
                                     start=True, stop=True)
                    t1 = wpool.tile([128, 512], dt.bfloat16, tag="pt1")
                    nc.vector.tensor_tensor(t1[:], u_sb[:], rb[:], AL.mult)
                    psB = mm_pool.tile([128, 512], dt.float32, tag="mm")
                    nc.tensor.matmul(psB[:], cw[pre + "wo1a"][:], t1[:],
                                     start=True, stop=False)
                    nc.tensor.matmul(psB[:], cw[pre + "wo1abar"][:], nmur[:],
                                     start=False, stop=False)
                    nc.tensor.matmul(psB[:], cw[pre + "wo1b"][:], rightT[:, sl],
                                     start=False, stop=True)
                    h2 = wpool.tile([128, 512], dt.bfloat16, tag="ph2")
                    nc.scalar.activation(h2[:], psB[:], LR, bias=cw[pre + "bo1"][:],
                                         alpha=SLOPE)
                    psC = mm_pool.tile([128, 512], dt.float32, tag="mm")
                    nc.tensor.matmul(psC[:], cw[pre + "wo2"][:], h2[:],
                                     start=True, stop=True)
                    nc.vector.tensor_scalar(outT[:, sl], psC[:], 1.0,
                                            cw[pre + "bo2"][:], AL.mult, AL.add)
                    for (wname, brow, dout) in projs:
                        for bi in range(4):
                            blo = lo + bi * 128
                            nv = min(128, max(0, n_valid - blo))
                            if nv == 0:
                                continue
                            psP = mm_pool.tile([128, 512], dt.float32, tag="mm")
                            nc.tensor.matmul(psP[:, :128], outT[:, blo : blo + 128],
                                             cw[wname][:], start=True, stop=True)
                            ob = wpool.tile([128, EMB], dt.bfloat16, tag="pob")
                            if brow is not None:
                                nc.vector.tensor_tensor(ob[:], psP[:, :128],
                                                        cw[brow][:], AL.add)
                            else:
                                nc.vector.tensor_copy(ob[:], psP[:, :128])
                            nc.sync.dma_start(b.d[dout][blo : blo + nv, :],
                                              ob[:nv, :])
                    if post_cb is not None:
                        post_cb(g)

                cur_rp = [None, -1]

                def get_rp(blk):
                    if cur_rp[1] == blk:
                        return cur_rp[0]
                    rp_sb = wpool.tile([128, EMB], dt.bfloat16, tag="rpblk")
                    lo = blk * 128
                    nv = min(128, n_valid - lo)
                    if nv < 128:
                        nc.vector.memset(rp_sb[:], 0.0)
                    nc.sync.dma_start(rp_sb[:nv, :], rp_dram[lo : lo + nv, :])
                    cur_rp[0] = rp_sb
                    cur_rp[1] = blk
                    return rp_sb

                for si, seg in enumerate(cv.segments):
                    blk = seg["blk"]
                    g = blk // 4
                    seg_first = (si == 0 or cv.segments[si - 1]["blk"] != blk)
                    seg_last = (si + 1 == len(cv.segments)
                                or cv.segments[si + 1]["blk"] != blk)
                    base_edge = seg["start_edge"]
                    ntiles = seg["ntiles"]
                    view_lo = HI_BASE if seg["u"] == 1 else 0
                    lp_view = lp_dram[view_lo:, :] if view_lo else lp_dram[:, :]
                    rp_sb = get_rp(blk)

                    tdone = 0
                    while tdone < ntiles:
                        tcn = min(CHUNK_TILES, ntiles - tdone)
                        e0 = base_edge + tdone * 128
                        ne = tcn * 128
                        sidx = gpool.tile([128, CHUNK_TILES * 8], dt.int16,
                                          tag="sidx")
                        nc.sync.dma_start(sidx[:, : ne // 16],
                                          src_d[:, e0 // 16 : (e0 + ne) // 16])
                        gbuf = gpool.tile([128, CHUNK_TILES, EMB], dt.bfloat16,
                                          tag="sgat")
                        nc.gpsimd.dma_gather(gbuf[:, :tcn, :], lp_view,
                                             sidx[:, : ne // 16], ne, ne, EMB,
                                             single_packet=False)
                        ohe = gpool.tile([128, CHUNK_TILES * 128], dt.float8e4,
                                         tag="ohe")
                        nc.sync.dma_start(ohe[:, :ne], oh_d[:, e0 : e0 + ne])
                        ohT = gpool.tile([128, CHUNK_TILES * 128], dt.float8e4,
                                         tag="ohT")
                        nc.sync.dma_start(ohT[:, :ne], ohT_d[:, e0 : e0 + ne])

                        xw_c = gpool.tile([128, CHUNK_TILES, EMB], dt.bfloat16,
                                          tag="xwc")
                        st6 = tpool.tile([128, CHUNK_TILES, 6], dt.float32,
                                         tag="st6")
                        mv_c = tpool.tile([128, CHUNK_TILES, 2], dt.float32,
                                          tag="mvc")

                        ngrp4 = -(-tcn // 4)
                        for g4 in range(ngrp4):
                            lo4 = g4 * 4
                            n4 = min(4, tcn - lo4)
                            psx = psx_pool.tile([128, 512], dt.float32, tag="psx")
                            for i in range(n4):
                                ti = lo4 + i
                                nc.tensor.matmul(
                                    psx[:, i * 128 : (i + 1) * 128],
                                    ohT[:, ti * 128 : (ti + 1) * 128], rp_sb[:],
                                    start=True, stop=True)
                            nc.vector.tensor_tensor(
                                xw_c[:, lo4 : lo4 + n4, :],
                                psx[:, : n4 * 128],
                                gbuf[:, lo4 : lo4 + n4, :], AL.add)
                            for i in range(n4):
                                ti = lo4 + i
                                nc.vector.bn_stats(st6[:, ti, :], xw_c[:, ti, :])
                                nc.vector.bn_aggr(mv_c[:, ti, :], st6[:, ti, :])

                        veps = tpool.tile([128, CHUNK_TILES], dt.float32,
                                          tag="vepsc")
                        nc.vector.tensor_scalar(veps[:, :tcn], mv_c[:, :tcn, 1],
                                                EPS, None, AL.add)
                        sdc = tpool.tile([128, CHUNK_TILES], dt.float32, tag="sdc")
                        nc.scalar.activation(sdc[:, :tcn], veps[:, :tcn], SQ)
                        rstd_t = tpool.tile([128, CHUNK_TILES], dt.float32,
                                            tag="rstdc")
                        nc.vector.reciprocal(rstd_t[:, :tcn], sdc[:, :tcn])
                        nmr_c = tpool.tile([128, CHUNK_TILES], dt.float32,
                                           tag="nmrc")
                        nc.vector.scalar_tensor_tensor(
                            nmr_c[:, :tcn], mv_c[:, :tcn, 0], -1.0,
                            rstd_t[:, :tcn], AL.mult, AL.mult)

                        for ti in range(tcn):
                            act = wpool.tile([128, EMB], dt.bfloat16, tag="act")
                            nc.scalar.activation(
                                act[:], xw_c[:, ti, :], LR,
                                bias=nmr_c[:, ti : ti + 1],
                                scale=rstd_t[:, ti : ti + 1], alpha=SLOPE)
                            if grp_ps[g] is None:
                                agg_t = agg_pool.tile([128, 512], dt.float32,
                                                      tag="agg")
                                grp_ps[g] = agg_t
                            first = seg_first and tdone == 0 and ti == 0
                            last = seg_last and (tdone + ti + 1 == ntiles)
                            bslot = blk % 4
                            nc.tensor.matmul(
                                grp_ps[g][:, bslot * 128 : (bslot + 1) * 128],
                                act[:], ohe[:, ti * 128 : (ti + 1) * 128],
                                start=first, stop=last)
                        tdone += tcn

                    if seg_last:
                        grp_done[g] += 1
                        gnb = min(4, nblocks - g * 4)
                        if grp_done[g] == gnb:
                            post_group(g)

                for g in range(ngroups):
                    gnb = min(4, nblocks - g * 4)
                    if grp_done[g] < gnb:
                        post_group(g)

            # =========== heads ===========
            def heads_chunk(j):
                sl = slice(j * 512, (j + 1) * 512)
                if nact == 0:
                    nc.vector.memset(outrow[:, sl], 0.0)
                    return
                pso = bc_pool.tile([128, 512], dt.float32, tag="bc")
                for hi in range(nact):
                    psH = mm_pool.tile([128, 512], dt.float32, tag="mm")
                    nc.tensor.matmul(psH[:], cw["hw1"][:, hi, :], v1T[:, sl],
                                     start=True, stop=True)
                    hh = wpool.tile([128, 512], dt.bfloat16, tag="hh")
                    nc.scalar.activation(hh[:], psH[:], LR,
                                         bias=cw["hb1"][:, hi : hi + 1],
                                         alpha=SLOPE)
                    nc.tensor.matmul(pso[0:1, :], cw["hw2"][:, hi : hi + 1], hh[:],
                                     start=(hi == 0), stop=(hi == nact - 1))
                nc.vector.tensor_scalar(outrow[:, sl], pso[0:1, :],
                                        p["out_scale"], p["out_add"],
                                        AL.mult, AL.add)

            # =========== run ===========
            if KSTAGE == "A":
                nc.vector.memset(outrow[:], 0.0)
            else:
                conv(c1p, "vc_", lp1_full, rp1_loc, b.d["e1_src"],
                     b.d["e1_oh"], b.d["e1_ohT"], c0T, c1T, "rcnt1", "ind1",
                     [("cv_wl", "cv_bl_row", "lp2_loc")], NCL)
                if KSTAGE == "C1":
                    nc.vector.memset(outrow[:], 0.0)
                else:
                    nc.gpsimd.collective_compute(
                        "AllGather", AL.bypass, ins=[lp2_loc[:]],
                        outs=[lp2_full[:]], replica_groups=[list(range(NCORES))])
                    heads_done = set()

                    def post2_cb(g):
                        if g not in heads_done:
                            heads_done.add(g)
                            heads_chunk(g)

                    conv(c2p, "cv_", lp2_full, rp2_loc, b.d["e2_src"],
                         b.d["e2_oh"], b.d["e2_ohT"], v0T, v1T, "rcnt2", "ind2",
                         [], NVL, post_cb=post2_cb)
                    for j in range(NVLg // 512):
                        if j not in heads_done:
                            heads_chunk(j)
            nc.sync.dma_start(out_d[:], outrow[:])

    nc.compile()
    return b


_CACHE = {}


def kernel(**inputs):
    key = tuple(sorted((k, tuple(np.asarray(v).shape)) for k, v in inputs.items()))
    p = host_prep(inputs)
    ck = (key, p["nact"], p["conv1"].etot, p["conv2"].etot)
    if ck in _CACHE:
        b = _CACHE[ck]
    else:
        b = build_program(p)
        _CACHE[ck] = b
    in_maps = [dict(p["core_inputs"][c]) for c in range(NCORES)]
    res = run_bass_kernel_spmd(b.nc, in_maps, core_ids=list(range(NCORES)))
    NVL = p["NVL"]
    out = np.concatenate([res.results[c]["out"][0, :NVL] for c in range(NCORES)])
    return out.astype(np.float32)
